# revision 1
# baseline (speedup 1.0000x reference)
"""Trainium2 Bass kernel for a belief-transformer block.

Computation (per batch b):
    h   = LayerNorm(x[b]) * g1
    qkv = h @ w_qkv ; q,k,v = split(qkv)
    s   = q @ k^T / sqrt(D), keys j >= L_b masked
    y   = softmax(s) @ v
    y   = LayerNorm(y) * g2
    out = gelu(y @ w_fc) @ w_proj

Sharding: data-parallel over batch across 8 NeuronCores (4 batches/core),
weights replicated.

Device-side structure per batch (tokens n, features d; P=128 partitions):
  A: LN1 stats+apply token-major          [n_chunk(128), d(512)]
  B: PE-transpose h -> h_T feature-major, interleaved with the v matmuls
     (key mask folded into the v copy-back)
  C: q_T,k_T feature-major
  then per token-half (512 queries) to bound SBUF:
  D: s_T = k @ q^T  (keys on partitions), exp on ACT -> p_T
  E: y_unnorm = p_T^T @ v_masked (token-major); rowsums r via a
     mask-column matmul into a [2, 512] PSUM row + PE-transpose shuffle
  F: LN2 on y_unnorm in-place; softmax normalization is absorbed by LN
     invariance, with the exact eps correction rsqrt(ssq/D + eps*r^2)
  G: PE-transpose y_ln -> y_ln_T, interleaved into neighboring matmul
     streams (PE transposes don't count as busy for the HAM clock gate)
  H: z_T = gelu(w_fc^T @ y_ln_T) feature-major
  I: out = z_T^T @ w_proj token-major -> DMA out

Matmuls run in float32r (full-rate, ~1e-4 rel err), accumulation fp32 in PSUM.
"""

import os
import sys

import numpy as np


def _ensure_concourse():
    try:
        import concourse  # noqa: F401
        return
    except ImportError:
        pass
    for p in ("/root/.axon_site/_ro/trn_rl_repo", "/opt/trn_rl_repo"):
        if os.path.isdir(p) and p not in sys.path:
            sys.path.insert(0, p)
    import concourse  # noqa: F401


_ensure_concourse()

import concourse.tile as tile  # noqa: E402
from concourse import bacc, mybir  # noqa: E402
from concourse.bass_utils import run_bass_kernel_spmd  # noqa: E402
from concourse.masks import make_identity  # noqa: E402

B, N, D = 32, 1024, 512
NCORES = 8
G = B // NCORES  # batches per core
P = 128
NT = N // P      # token chunks (8)
DC = D // P      # feature chunks (4)
HT_ = N // 2     # token half (512)
HC = HT_ // P    # token chunks per half (4)
EPS = 1e-5

F32 = mybir.dt.float32
F32R = mybir.dt.float32r
ALU = mybir.AluOpType
ACT = mybir.ActivationFunctionType


def _body(ctx, tc, x, msk, wdram, out, warm):
    nc = tc.nc

    singles = ctx.enter_context(tc.tile_pool(name="singles", bufs=1))
    main = ctx.enter_context(tc.tile_pool(name="main", bufs=1))
    xpool = ctx.enter_context(tc.tile_pool(name="xpool", bufs=2))
    outp = ctx.enter_context(tc.tile_pool(name="outp", bufs=3))
    stats = ctx.enter_context(tc.tile_pool(name="stats", bufs=2))
    ps_mm = ctx.enter_context(tc.tile_pool(name="ps_mm", bufs=4, space="PSUM"))
    ps_t = ctx.enter_context(tc.tile_pool(name="ps_t", bufs=3, space="PSUM"))
    ps_r = ctx.enter_context(tc.tile_pool(name="ps_r", bufs=1, space="PSUM"))

    # Replicated weights, feature-chunked [P, DC, D]; float32r for the PE.
    # wv is loaded first (feeds the PE warm-up and the earliest matmuls);
    # batch 0's x/mask DMAs are emitted before the remaining weights so the
    # LN1 critical path is not queued behind 5 MB of weight traffic.
    W = {}

    def load_w(name):
        t = singles.tile([P, DC, D], F32R, tag=name, name=name)
        nc.sync.dma_start(
            t[:], wdram[name].rearrange("(c p) e -> p c e", p=P).bitcast(F32R)
        )
        W[name] = t

    ident = singles.tile([P, P], F32, tag="ident")
    make_identity(nc, ident)
    eps_t = singles.tile([P, 1], F32, tag="eps")
    nc.vector.memset(eps_t[:], EPS)

    S = [dict() for _ in range(G)]  # per-batch live tiles

    def emit_A(b):
        """Load x/mask, LN1 -> H (DVE/ACT only)."""
        s = S[b]
        xb = x[b].rearrange("(t p) d -> p t d", p=P)
        mb = msk[b].rearrange("(t p) -> p t", p=P)
        X = xpool.tile([P, NT, D], F32, tag="X")
        nc.sync.dma_start(X[:, 0:2, :], xb[:, 0:2, :])
        nc.sync.dma_start(X[:, 2:NT, :], xb[:, 2:NT, :])
        s["mask_f"] = xpool.tile([P, NT], F32, tag="mask_f", name="mask_f")
        nc.sync.dma_start(s["mask_f"][:], mb)
        mask_s = xpool.tile([P, NT], F32R, tag="mask_s")
        nc.sync.dma_start(mask_s[:], mb.bitcast(F32R))
        s["mask_r"] = xpool.tile([P, NT, 2], F32R, tag="mask_r", name="mask_r")
        nc.vector.tensor_copy(
            s["mask_r"][:], mask_s[:, :, None].to_broadcast((P, NT, 2))
        )
        s["H"] = main.tile([P, NT, D], F32, tag="H", name="H")
        for t in range(NT):
            st = stats.tile([P, 6], F32, tag="bnst")
            nc.vector.bn_stats(st[:], X[:, t, :])
            mv = stats.tile([P, 2], F32, tag="bnag")
            nc.vector.bn_aggr(mv[:], st[:])
            sd = stats.tile([P, 1], F32, tag="sd")
            nc.scalar.activation(sd[:], mv[:, 1:2], ACT.Sqrt, bias=eps_t[:])
            rstd = stats.tile([P, 1], F32, tag="rstd")
            nc.vector.reciprocal(rstd[:], sd[:])
            nc.vector.tensor_scalar(
                s["H"][:, t, :], X[:, t, :], mv[:, 0:1], rstd[:],
                op0=ALU.subtract, op1=ALU.mult,
            )

    def emit_B_alloc(b):
        s = S[b]
        s["HT"] = main.tile([P, DC, N], F32R, tag="HT", name="HT")
        s["VM"] = main.tile([P, NT, D], F32R, tag="VM", name="VM")

    def emit_Bt(b, t):
        """Transpose h chunk t -> h_T, and the v-matmul for chunk t
        (interleaves real matmuls into the transpose burst for HAM)."""
        s = S[b]
        pt = ps_t.tile([P, DC, P], F32, tag="pst")
        for c in range(DC):
            nc.tensor.transpose(
                pt[:, c, :], s["H"][:, t, c * P:(c + 1) * P], ident[:]
            )
        nc.vector.tensor_copy(s["HT"][:, :, t * P:(t + 1) * P], pt[:])
        pm = ps_mm.tile([P, 512], F32, tag="psmm")
        for dc_ in range(DC):
            nc.tensor.matmul(
                pm[:],
                s["HT"][:, dc_, t * P:(t + 1) * P],
                W["wv"][:, dc_, :],
                start=(dc_ == 0), stop=(dc_ == DC - 1),
            )
        # mask keys >= L_b by zeroing their v rows during the copy-back
        nc.vector.tensor_scalar_mul(s["VM"][:, t, :], pm[:], s["mask_f"][:, t:t + 1])

    def emit_C_alloc(b):
        s = S[b]
        s["QT"] = main.tile([P, DC, N], F32R, tag="QT", name="QT")
        s["KT"] = main.tile([P, DC, N], F32R, tag="KT", name="KT")

    def emit_C_piece(b, h, c, which):
        """One PSUM group of the q_T/k_T production (4 matmuls)."""
        s = S[b]
        wt, tt = ((W["wq"], s["QT"]), (W["wk"], s["KT"]))[which]
        pm = ps_mm.tile([P, 512], F32, tag="psmm")
        for dc_ in range(DC):
            nc.tensor.matmul(
                pm[:],
                wt[:, dc_, c * P:(c + 1) * P],
                s["HT"][:, dc_, h * 512:(h + 1) * 512],
                start=(dc_ == 0), stop=(dc_ == DC - 1),
            )
        nc.scalar.copy(tt[:, c, h * 512:(h + 1) * 512], pm[:])

    def emit_C_half(b, h):
        for which in range(2):
            for c in range(DC):
                emit_C_piece(b, h, c, which)

    def emit_D_alloc(b, hf):
        S[b][f"PT{hf}"] = main.tile([P, NT, HT_], F32R, tag="PT", name="PT")

    def emit_D(b, hf, jc):
        """Scores for key-chunk jc (keys on partitions) + exp."""
        s = S[b]
        q0 = hf * HT_
        pm = ps_mm.tile([P, 512], F32, tag="psmm")
        for dc_ in range(DC):
            nc.tensor.matmul(
                pm[:],
                s["KT"][:, dc_, jc * P:(jc + 1) * P],
                s["QT"][:, dc_, q0:q0 + HT_],
                start=(dc_ == 0), stop=(dc_ == DC - 1),
            )
        nc.scalar.activation(s[f"PT{hf}"][:, jc, :], pm[:], ACT.Exp)

    def emit_E(b, hf):
        """y_unnorm = p^T @ v_masked; rowsums r via mask-column matmuls."""
        s = S[b]
        PT = s[f"PT{hf}"]
        Y = main.tile([P, HC, D], F32, tag="Y", name="Y")
        ysum = stats.tile([P, HC], F32, tag="ysum", name="ysum")
        R = stats.tile([P, HC], F32, tag="R", name="R")
        s[f"Y{hf}"], s[f"ysum{hf}"], s[f"R{hf}"] = Y, ysum, R
        pr2 = ps_r.tile([2, HT_], F32, tag="psr2", name="psr2")
        for jc in range(NT):
            nc.tensor.matmul(
                pr2[:],
                s["mask_r"][:, jc, :],
                PT[:, jc, :],
                start=(jc == 0), stop=(jc == NT - 1),
            )
        rrow = stats.tile([2, HT_], F32, tag="rrow", name="rrow")
        nc.vector.tensor_copy(rrow[:], pr2[:])
        for il in range(HC):
            pm = ps_mm.tile([P, 512], F32, tag="psmm")
            for jc in range(NT):
                nc.tensor.matmul(
                    pm[:],
                    PT[:, jc, il * P:(il + 1) * P],
                    s["VM"][:, jc, :],
                    start=(jc == 0), stop=(jc == NT - 1),
                )
            nc.scalar.activation(
                Y[:, il, :], pm[:], ACT.Copy, accum_out=ysum[:, il:il + 1]
            )
        # cross-partition shuffle [1, 512] -> [128, HC] via PE transposes,
        # after the y-matmuls so the PE never waits on the rrow copy
        for c in range(HC):
            ptr = ps_r.tile([P, 2], F32, tag="psr2", name="ptr")
            nc.tensor.transpose(
                ptr[:], rrow[:, c * P:(c + 1) * P], ident[0:2, 0:2]
            )
            nc.vector.tensor_copy(R[:, c:c + 1], ptr[:, 0:1])

    def emit_F(b, hf):
        """LN2 in-place on Y (absorbs softmax norm; exact eps via r^2).
        ACT functions grouped (all Squares, then all Sqrts) to minimize
        activation-table reloads."""
        s = S[b]
        Y, ysum, R = s[f"Y{hf}"], s[f"ysum{hf}"], s[f"R{hf}"]
        negmus, ssqs, sds = [], [], []
        for il in range(HC):
            negmu = stats.tile([P, 1], F32, tag="negmu")
            nc.vector.tensor_scalar(
                negmu[:], ysum[:, il:il + 1], -1.0 / D, None, op0=ALU.mult
            )
            negmus.append(negmu)
        for il in range(HC):
            sq = stats.tile([P, D], F32, tag="sq")
            ssq = stats.tile([P, 1], F32, tag="ssq")
            nc.scalar.activation(
                sq[:], Y[:, il, :], ACT.Square, bias=negmus[il][:],
                accum_out=ssq[:],
            )
            ssqs.append(ssq)
        for il in range(HC):
            epsr2 = stats.tile([P, 1], F32, tag="epsr2")
            nc.vector.tensor_tensor(
                epsr2[:], R[:, il:il + 1], R[:, il:il + 1], ALU.mult
            )
            nc.vector.tensor_scalar(
                epsr2[:], epsr2[:], EPS, None, op0=ALU.mult
            )
            sd2 = stats.tile([P, 1], F32, tag="sd2")
            nc.scalar.activation(
                sd2[:], ssqs[il][:], ACT.Sqrt, bias=epsr2[:], scale=1.0 / D
            )
            sds.append(sd2)
        for il in range(HC):
            rstd2 = stats.tile([P, 1], F32, tag="rstd2")
            nc.vector.reciprocal(rstd2[:], sds[il][:])
            nc.vector.tensor_scalar(
                Y[:, il, :], Y[:, il, :], negmus[il][:], rstd2[:],
                op0=ALU.add, op1=ALU.mult,
            )

    def emit_G_alloc(b, hf):
        S[b][f"YLT{hf}"] = main.tile([P, DC, HT_], F32R, tag="YLT", name="YLT")

    def emit_G_pair(b, hf, k):
        """Two of the 16 y_ln transposes (k in 0..7)."""
        s = S[b]
        Y, YLT = s[f"Y{hf}"], s[f"YLT{hf}"]
        tl, c0 = divmod(2 * k, DC)
        pt = ps_t.tile([P, 2, P], F32, tag="pst")
        for j in range(2):
            nc.tensor.transpose(
                pt[:, j, :], Y[:, tl, (c0 + j) * P:(c0 + j + 1) * P], ident[:]
            )
        nc.vector.tensor_copy(YLT[:, c0:c0 + 2, tl * P:(tl + 1) * P], pt[:])

    def emit_HI(b, hf):
        """fc + gelu feature-major, then proj token-major + store."""
        s = S[b]
        YLT = s[f"YLT{hf}"]
        ob = out[b].rearrange("(t p) d -> p t d", p=P)
        ZT = main.tile([P, DC, HT_], F32R, tag="ZT", name="ZT")
        for c in range(DC):
            pm = ps_mm.tile([P, 512], F32, tag="psmm")
            for ec in range(DC):
                nc.tensor.matmul(
                    pm[:],
                    W["wf"][:, ec, c * P:(c + 1) * P],
                    YLT[:, ec, :],
                    start=(ec == 0), stop=(ec == DC - 1),
                )
            nc.scalar.activation(ZT[:, c, :], pm[:], ACT.Gelu)
        for il in range(HC):
            pm = ps_mm.tile([P, 512], F32, tag="psmm")
            for c in range(DC):
                nc.tensor.matmul(
                    pm[:],
                    ZT[:, c, il * P:(il + 1) * P],
                    W["wp"][:, c, :],
                    start=(c == 0), stop=(c == DC - 1),
                )
            o = outp.tile([P, D], F32, tag="O")
            nc.vector.tensor_copy(o[:], pm[:])
            nc.sync.dma_start(ob[:, hf * HC + il, :], o[:])

    # --- batch pipeline with transposes interleaved into matmul streams ---
    a_done = [False] * G
    bc_done = [False] * G
    for b in range(G):
        if b == 0:
            # startup: x/mask first, then wv + PE warm-up, then the rest
            load_w("wv")
            emit_A(0)

            def warm_burst(k0, n_mm, last):
                wpm = ps_mm.tile([P, 512], F32, tag="psmm", name="warmmm")
                for k in range(n_mm):
                    nc.tensor.matmul(
                        wpm[:], W["wv"][:, (k0 + k) % DC, 0:P],
                        W["wv"][:, (k0 + k) % DC, :],
                        start=(k == 0), stop=(k == n_mm - 1),
                    )
                if last:
                    wsb = outp.tile([P, 8], F32, tag="O", name="warmsb")
                    nc.vector.tensor_copy(wsb[:], wpm[:, 0:8])
                    nc.sync.dma_start(warm[:], wsb[:])

            warm_burst(0, 10, False)
            for name in ("wq", "wk", "wf", "wp"):
                load_w(name)
            a_done[0] = True
            emit_B_alloc(0)
            emit_C_alloc(0)
            for t in range(NT):
                emit_Bt(0, t)
                if t == 0:
                    warm_burst(10, 8, False)
                elif t == 1:
                    warm_burst(18, 8, True)
                if t >= HC:
                    emit_C_piece(0, 0, t - HC, 0)
                    emit_C_piece(0, 0, t - HC, 1)
            emit_C_half(0, 1)
            bc_done[0] = True
        # half 0: scores
        emit_D_alloc(b, 0)
        for jc in range(NT):
            emit_D(b, 0, jc)
        emit_E(b, 0)
        emit_F(b, 0)
        # half 1 scores interleaved with half-0 y_ln transposes
        emit_D_alloc(b, 1)
        emit_G_alloc(b, 0)
        if b + 1 < G:
            emit_A(b + 1)  # DVE work, overlaps the D(h1)/E(h1) PE stream
            a_done[b + 1] = True
        for jc in range(NT):
            emit_D(b, 1, jc)
            if jc >= HC:
                emit_G_pair(b, 0, 2 * (jc - HC))
                emit_G_pair(b, 0, 2 * (jc - HC) + 1)
        emit_HI(b, 0)
        emit_E(b, 1)
        emit_F(b, 1)
        emit_G_alloc(b, 1)
        if b + 1 < G:
            # tail: next batch's transposes + v/qk matmuls + this G(h1)
            emit_B_alloc(b + 1)
            emit_C_alloc(b + 1)
            for t in range(NT):
                emit_Bt(b + 1, t)
                if t >= HC:
                    emit_C_piece(b + 1, 0, t - HC, 0)
                    emit_C_piece(b + 1, 0, t - HC, 1)
            for k2 in range(NT):
                emit_G_pair(b, 1, k2)
                emit_C_piece(b + 1, 1, k2 % DC, k2 // DC)
            bc_done[b + 1] = True
        else:
            for k2 in range(NT):
                emit_G_pair(b, 1, k2)
        emit_HI(b, 1)


def build():
    from contextlib import ExitStack

    nc = bacc.Bacc("TRN2", target_bir_lowering=False, debug=False,
                   num_devices=NCORES)
    x = nc.dram_tensor("x", [G, N, D], F32, kind="ExternalInput").ap()
    msk = nc.dram_tensor("msk", [G, N], F32, kind="ExternalInput").ap()
    wdram = {
        name: nc.dram_tensor(name, [D, D], F32, kind="ExternalInput").ap()
        for name in ("wq", "wk", "wv", "wf", "wp")
    }
    out = nc.dram_tensor("out", [G, N, D], F32, kind="ExternalOutput").ap()
    warm = nc.dram_tensor("warm", [P, 8], F32, kind="ExternalOutput").ap()

    with tile.TileContext(nc) as tc:
        with ExitStack() as ctx:
            _body(ctx, tc, x, msk, wdram, out, warm)
    nc.compile()
    return nc


_NC_CACHE = None


def get_nc():
    global _NC_CACHE
    if _NC_CACHE is None:
        _NC_CACHE = build()
    return _NC_CACHE


def make_in_maps(x, belief_base_sizes, g1, w_qkv, g2, w_fc, w_proj):
    x = np.asarray(x, dtype=np.float32)
    sizes = np.asarray(belief_base_sizes, dtype=np.int64)
    g1 = np.asarray(g1, dtype=np.float32)
    w_qkv = np.asarray(w_qkv, dtype=np.float32)
    g2 = np.asarray(g2, dtype=np.float32)
    w_fc = np.asarray(w_fc, dtype=np.float32)
    w_proj = np.asarray(w_proj, dtype=np.float32)

    wq = np.ascontiguousarray((g1[:, None] * w_qkv[:, :D]) / np.float32(np.sqrt(D)))
    wk = np.ascontiguousarray(g1[:, None] * w_qkv[:, D:2 * D])
    wv = np.ascontiguousarray(g1[:, None] * w_qkv[:, 2 * D:])
    wf = np.ascontiguousarray(g2[:, None] * w_fc)
    wp = np.ascontiguousarray(w_proj)

    mask = (np.arange(N)[None, :] < sizes[:, None]).astype(np.float32)  # [B, N]

    in_maps = []
    for c in range(NCORES):
        sl = slice(c * G, (c + 1) * G)
        in_maps.append({
            "x": np.ascontiguousarray(x[sl]),
            "msk": np.ascontiguousarray(mask[sl]),
            "wq": wq, "wk": wk, "wv": wv, "wf": wf, "wp": wp,
        })
    return in_maps


def kernel(x, belief_base_sizes, g1, w_qkv, g2, w_fc, w_proj):
    in_maps = make_in_maps(x, belief_base_sizes, g1, w_qkv, g2, w_fc, w_proj)
    nc = get_nc()
    res = run_bass_kernel_spmd(nc, in_maps, core_ids=list(range(NCORES)))
    out = np.concatenate([res.results[c]["out"] for c in range(NCORES)], axis=0)
    return np.ascontiguousarray(out.astype(np.float32))



# revision 8
# speedup vs baseline: 1.6023x; 1.6023x over previous
"""Trainium2 Bass kernel for a belief-transformer block (sparse attention).

Computation (per batch b):
    h   = LayerNorm(x[b]) * g1
    qkv = h @ w_qkv ; q,k,v = split(qkv)
    s   = q @ k^T / sqrt(D), keys j >= L_b masked
    y   = softmax(s) @ v
    y   = LayerNorm(y) * g2
    out = gelu(y @ w_fc) @ w_proj

Sharding: data-parallel over batch across 8 NeuronCores (4 slot-batches per
core), weights replicated.  Sparsity: the host sorts batches by
nkc_b = ceil(L_b/128) (number of live 128-key chunks), assigns rank
8s+c to core c slot s, and compiles the program for per-slot chunk
maxima kcs[s] = max over cores.  Key chunks >= kcs[s] are skipped
entirely (k/v production, scores, exp, attn@v); chunks < kcs[s] but
>= the batch's own nkc are masked exactly via the exp bias
(exp(s + log_mask), log_mask in {0, -1e30} per key partition).

Engine layout:
  PE:     transposes (bf16), all matmuls (bf16 operands, fp32 PSUM)
  ACT:    LN applies (Identity w/ scale+bias), exp (with log-mask bias),
          gelu, Y copies; table sets touched only 4x total
          (Sqrt -> Exp -> Sqrt -> Gelu)
  DVE:    bn_stats/aggr for both LNs, psum->SBUF copies, LN2 apply
  Softmax normalization is absorbed into LN2 (scale invariance), with the
  exact eps correction sd = sqrt(var_u + eps*r^2), r = sum_k exp(s).

Phases: 0) LN1 stats for all batches  A) attention per batch (exp set only)
        B) LN2 rstd for all (sqrt set), then apply/transpose/MLP (gelu set).
"""

import os
import sys

import numpy as np

try:
    import ml_dtypes
except ImportError:  # pragma: no cover
    ml_dtypes = None


def _ensure_concourse():
    try:
        import concourse  # noqa: F401
        return
    except ImportError:
        pass
    for p in ("/root/.axon_site/_ro/trn_rl_repo", "/opt/trn_rl_repo"):
        if os.path.isdir(p) and p not in sys.path:
            sys.path.insert(0, p)
    import concourse  # noqa: F401


_ensure_concourse()

import concourse.tile as tile  # noqa: E402
from concourse import bacc, mybir  # noqa: E402
from concourse.bass_utils import run_bass_kernel_spmd  # noqa: E402
from concourse.masks import make_identity  # noqa: E402

B, N, D = 32, 1024, 512
NCORES = 8
G = B // NCORES  # batches (slots) per core
P = 128
NT = N // P      # token chunks (8)
DC = D // P      # feature chunks (4)
HT_ = N // 2     # token half (512)
HC = HT_ // P    # token chunks per half (4)
EPS = 1e-5
NEG = -1e30

F32 = mybir.dt.float32
BF16 = mybir.dt.bfloat16
ALU = mybir.AluOpType
ACT = mybir.ActivationFunctionType


def _body(ctx, tc, x, lb, wdram, out, warm, kcs):
    nc = tc.nc

    singles = ctx.enter_context(tc.tile_pool(name="singles", bufs=1))
    main = ctx.enter_context(tc.tile_pool(name="main", bufs=1))
    vmp = ctx.enter_context(tc.tile_pool(name="vmp", bufs=2))
    work = ctx.enter_context(tc.tile_pool(name="work", bufs=2))
    outp = ctx.enter_context(tc.tile_pool(name="outp", bufs=3))
    stats = ctx.enter_context(tc.tile_pool(name="stats", bufs=2))
    ps_mm = ctx.enter_context(tc.tile_pool(name="ps_mm", bufs=4, space="PSUM"))
    ps_t = ctx.enter_context(tc.tile_pool(name="ps_t", bufs=2, space="PSUM"))
    ps_r = ctx.enter_context(tc.tile_pool(name="ps_r", bufs=2, space="PSUM"))

    # ---- replicated weights, feature-chunked [P, DC, D] bf16 ----
    W = {}

    def load_w(name):
        t = singles.tile([P, DC, D], BF16, tag=name, name=name)
        nc.sync.dma_start(t[:], wdram[name].rearrange("(c p) e -> p c e", p=P))
        W[name] = t

    ident = singles.tile([P, P], BF16, tag="ident")
    make_identity(nc, ident)
    ones = singles.tile([P, 1], BF16, tag="ones")
    nc.vector.memset(ones[:], 1.0)
    eps_t = singles.tile([P, 1], F32, tag="eps")
    nc.vector.memset(eps_t[:], EPS)

    # persistent per-batch state
    X = singles.tile([P, G, NT, D], BF16, tag="X", name="X")
    LB = singles.tile([P, G, NT], F32, tag="LB", name="LB")
    YS = singles.tile([P, G, 2, HC, D], BF16, tag="YS", name="YS")
    RS1 = singles.tile([P, G, NT], F32, tag="RS1", name="RS1")
    NM1 = singles.tile([P, G, NT], F32, tag="NM1", name="NM1")
    MV2 = singles.tile([P, G, 2, HC, 2], F32, tag="MV2", name="MV2")
    RALL = singles.tile([P, G, 2, HC], F32, tag="RALL", name="RALL")
    RSTD2 = singles.tile([P, G, 2, HC], F32, tag="RSTD2", name="RSTD2")
    NM2 = singles.tile([P, G, 2, HC], F32, tag="NM2", name="NM2")

    S = [dict() for _ in range(G)]  # per-batch transient tiles

    def dma_in(b):
        xb = x[b].rearrange("(t p) d -> p t d", p=P)
        nc.sync.dma_start(X[:, b, :, :], xb)
        nc.sync.dma_start(LB[:, b, :], lb[b].rearrange("(t p) -> p t", p=P))

    def ln1_stats(b):
        """Phase 0: bn over X, rstd1/negmu1 (DVE + scalar Sqrt)."""
        for t in range(NT):
            st = stats.tile([P, 6], F32, tag="bnst")
            nc.vector.bn_stats(st[:], X[:, b, t, :])
            mv = stats.tile([P, 2], F32, tag="bnag")
            nc.vector.bn_aggr(mv[:], st[:])
            sd = stats.tile([P, 1], F32, tag="sd")
            nc.scalar.activation(sd[:], mv[:, 1:2], ACT.Sqrt, bias=eps_t[:])
            nc.vector.reciprocal(RS1[:, b, t:t + 1], sd[:])
            nc.vector.tensor_scalar(
                NM1[:, b, t:t + 1], mv[:, 0:1], RS1[:, b, t:t + 1], -1.0,
                op0=ALU.mult, op1=ALU.mult,
            )

    def emit_H(b):
        """LN1 apply on scalar: H = Identity(rstd*x + (-mu*rstd)) bf16."""
        s = S[b]
        s["H"] = main.tile([P, NT, D], BF16, tag="H", name="H")
        for t in range(NT):
            nc.scalar.activation(
                s["H"][:, t, :], X[:, b, t, :], ACT.Identity,
                bias=NM1[:, b, t:t + 1], scale=RS1[:, b, t:t + 1],
            )

    def emit_B_alloc(b):
        s = S[b]
        s["HT"] = main.tile([P, DC, N], BF16, tag="HT", name="HT")
        s["VM"] = vmp.tile([P, NT, D], BF16, tag="VM", name="VM")

    def emit_Bt(b, t, kc):
        """Transpose h chunk t -> h_T; v-matmul for live chunks."""
        s = S[b]
        pt = ps_t.tile([P, DC, P], BF16, tag="pst")
        for c in range(DC):
            nc.tensor.transpose(
                pt[:, c, :], s["H"][:, t, c * P:(c + 1) * P], ident[:]
            )
        nc.vector.tensor_copy(s["HT"][:, :, t * P:(t + 1) * P], pt[:])
        if t < kc:
            pm = ps_mm.tile([P, 512], F32, tag="psmm")
            for dc_ in range(DC):
                nc.tensor.matmul(
                    pm[:],
                    s["HT"][:, dc_, t * P:(t + 1) * P],
                    W["wv"][:, dc_, :],
                    start=(dc_ == 0), stop=(dc_ == DC - 1),
                )
            nc.vector.tensor_copy(s["VM"][:, t, :], pm[:])

    def emit_C_alloc(b):
        s = S[b]
        s["QT"] = main.tile([P, DC, N], BF16, tag="QT", name="QT")
        s["KT"] = main.tile([P, DC, N], BF16, tag="KT", name="KT")

    def emit_Q_piece(b, h, c):
        """One PSUM group of q_T production (4 matmuls, 512 tokens)."""
        s = S[b]
        pm = ps_mm.tile([P, 512], F32, tag="psmm")
        for dc_ in range(DC):
            nc.tensor.matmul(
                pm[:],
                W["wq"][:, dc_, c * P:(c + 1) * P],
                s["HT"][:, dc_, h * 512:(h + 1) * 512],
                start=(dc_ == 0), stop=(dc_ == DC - 1),
            )
        nc.scalar.activation(
            s["QT"][:, c, h * 512:(h + 1) * 512], pm[:], ACT.Identity
        )

    def emit_K_span(b, h, c):
        """k_T production for a full 512-key span."""
        s = S[b]
        pm = ps_mm.tile([P, 512], F32, tag="psmm")
        for dc_ in range(DC):
            nc.tensor.matmul(
                pm[:],
                W["wk"][:, dc_, c * P:(c + 1) * P],
                s["HT"][:, dc_, h * 512:(h + 1) * 512],
                start=(dc_ == 0), stop=(dc_ == DC - 1),
            )
        nc.vector.tensor_copy(s["KT"][:, c, h * 512:(h + 1) * 512], pm[:])

    def emit_K_chunk(b, jc, c):
        """k_T production for one 128-key chunk (remainder)."""
        s = S[b]
        pk = ps_mm.tile([P, 512], F32, tag="psmm", name="psk")
        for dc_ in range(DC):
            nc.tensor.matmul(
                pk[:, 0:P],
                W["wk"][:, dc_, c * P:(c + 1) * P],
                s["HT"][:, dc_, jc * P:(jc + 1) * P],
                start=(dc_ == 0), stop=(dc_ == DC - 1),
            )
        nc.vector.tensor_copy(s["KT"][:, c, jc * P:(jc + 1) * P], pk[:, 0:P])

    def emit_front(b, kc):
        """H, transposes+v, q, k for batch b (no scalar exp work)."""
        emit_H(b)
        emit_B_alloc(b)
        emit_C_alloc(b)
        for t in range(NT):
            emit_Bt(b, t, kc)
        for h in range(2):
            for c in range(DC):
                emit_Q_piece(b, h, c)
        for h in range(kc // 4):
            for c in range(DC):
                emit_K_span(b, h, c)
        for jc in range(4 * (kc // 4), kc):
            for c in range(DC):
                emit_K_chunk(b, jc, c)

    def emit_D_alloc(b, hf):
        S[b][f"PT{hf}"] = main.tile([P, NT, HT_], BF16, tag=f"PT{hf}",
                                    name="PT")

    def emit_D(b, hf, jc):
        """Scores for key-chunk jc (keys on partitions) + masked exp."""
        s = S[b]
        q0 = hf * HT_
        pm = ps_mm.tile([P, 512], F32, tag="psmm")
        for dc_ in range(DC):
            nc.tensor.matmul(
                pm[:],
                s["KT"][:, dc_, jc * P:(jc + 1) * P],
                s["QT"][:, dc_, q0:q0 + HT_],
                start=(dc_ == 0), stop=(dc_ == DC - 1),
            )
        nc.scalar.activation(
            s[f"PT{hf}"][:, jc, :], pm[:], ACT.Exp, bias=LB[:, b, jc:jc + 1]
        )

    def emit_E(b, hf, kc):
        """y_unnorm = p^T @ v; rowsums r via 1-col matmuls; LN2 stats."""
        s = S[b]
        PT = s[f"PT{hf}"]
        for il in range(HC):
            pm = ps_mm.tile([P, 512], F32, tag="psmm")
            pr = ps_r.tile([P, 1], F32, tag="psr", name="pr")
            for jc in range(kc):
                nc.tensor.matmul(
                    pm[:],
                    PT[:, jc, il * P:(il + 1) * P],
                    s["VM"][:, jc, :],
                    start=(jc == 0), stop=(jc == kc - 1),
                )
                nc.tensor.matmul(
                    pr[:],
                    PT[:, jc, il * P:(il + 1) * P],
                    ones[:],
                    start=(jc == 0), stop=(jc == kc - 1),
                )
            st = stats.tile([P, 6], F32, tag="bnst")
            nc.vector.bn_stats(st[:], pm[:])
            nc.vector.bn_aggr(MV2[:, b, hf, il, :], st[:])
            nc.scalar.activation(YS[:, b, hf, il, :], pm[:], ACT.Identity)
            nc.vector.tensor_copy(RALL[:, b, hf, il:il + 1], pr[:])

    def emit_rstd2(b, hf):
        """Phase B head: rstd2 = 1/sqrt(var_u + eps*r^2), negmu2."""
        for il in range(HC):
            epsr2 = stats.tile([P, 1], F32, tag="epsr2")
            nc.vector.scalar_tensor_tensor(
                epsr2[:], RALL[:, b, hf, il:il + 1], EPS,
                RALL[:, b, hf, il:il + 1], op0=ALU.mult, op1=ALU.mult,
            )
            sd2 = stats.tile([P, 1], F32, tag="sd2")
            nc.scalar.activation(
                sd2[:], MV2[:, b, hf, il, 1:2], ACT.Sqrt, bias=epsr2[:]
            )
            nc.vector.reciprocal(RSTD2[:, b, hf, il:il + 1], sd2[:])
            nc.vector.tensor_scalar(
                NM2[:, b, hf, il:il + 1], MV2[:, b, hf, il, 0:1],
                RSTD2[:, b, hf, il:il + 1], -1.0,
                op0=ALU.mult, op1=ALU.mult,
            )

    def emit_tail(b, hf):
        """LN2 apply, transpose, fc+gelu, proj, store (gelu set only)."""
        YB = work.tile([P, HC, D], BF16, tag="YB", name="YB")
        for il in range(HC):
            nc.vector.tensor_scalar(
                YB[:, il, :], YS[:, b, hf, il, :],
                RSTD2[:, b, hf, il:il + 1], NM2[:, b, hf, il:il + 1],
                op0=ALU.mult, op1=ALU.add,
            )
        YLT = work.tile([P, DC, HT_], BF16, tag="YLT", name="YLT")
        for tl in range(HC):
            pt = ps_t.tile([P, DC, P], BF16, tag="pst")
            for c in range(DC):
                nc.tensor.transpose(
                    pt[:, c, :], YB[:, tl, c * P:(c + 1) * P], ident[:]
                )
            nc.vector.tensor_copy(YLT[:, :, tl * P:(tl + 1) * P], pt[:])
        ZT = work.tile([P, DC, HT_], BF16, tag="ZT", name="ZT")
        for c in range(DC):
            pm = ps_mm.tile([P, 512], F32, tag="psmm")
            for ec in range(DC):
                nc.tensor.matmul(
                    pm[:],
                    W["wf"][:, ec, c * P:(c + 1) * P],
                    YLT[:, ec, :],
                    start=(ec == 0), stop=(ec == DC - 1),
                )
            nc.scalar.activation(ZT[:, c, :], pm[:], ACT.Gelu)
        ob = out[b].rearrange("(t p) d -> p t d", p=P)
        for il in range(HC):
            pm = ps_mm.tile([P, 512], F32, tag="psmm")
            for c in range(DC):
                nc.tensor.matmul(
                    pm[:],
                    ZT[:, c, il * P:(il + 1) * P],
                    W["wp"][:, c, :],
                    start=(c == 0), stop=(c == DC - 1),
                )
            o = outp.tile([P, D], F32, tag="O")
            nc.scalar.activation(o[:], pm[:], ACT.Identity)
            nc.sync.dma_start(ob[:, hf * HC + il, :], o[:])

    # ---------------- emission ----------------
    # startup: wv + batch DMAs first, PE warm-up, remaining weights
    load_w("wv")
    for b in range(G):
        dma_in(b)

    def warm_burst(k0, n_mm, last):
        wpm = ps_mm.tile([P, 512], F32, tag="psmm", name="warmmm")
        for k in range(n_mm):
            nc.tensor.matmul(
                wpm[:], W["wv"][:, (k0 + k) % DC, 0:P],
                W["wv"][:, (k0 + k) % DC, :],
                start=(k == 0), stop=(k == n_mm - 1),
            )
        if last:
            wsb = outp.tile([P, 8], F32, tag="O", name="warmsb")
            nc.vector.tensor_copy(wsb[:], wpm[:, 0:8])
            nc.sync.dma_start(warm[:], wsb[:])

    warm_burst(0, 10, False)
    for name in ("wq", "wk", "wf", "wp"):
        load_w(name)

    # phase 0: LN1 stats (sqrt set); PE warm bursts keep HAM busy
    ln1_stats(0)
    warm_burst(10, 8, False)
    for b in range(1, G):
        ln1_stats(b)
    warm_burst(18, 8, True)

    # phase A
    emit_front(0, kcs[0])
    for b in range(G):
        kc = kcs[b]
        emit_D_alloc(b, 0)
        emit_D_alloc(b, 1)
        for jc in range(kc):
            emit_D(b, 0, jc)
        for jc in range(kc):
            emit_D(b, 1, jc)
        emit_E(b, 0, kc)
        if b + 1 < G:
            emit_front(b + 1, kcs[b + 1])
        emit_E(b, 1, kc)

    # phase B: all rstd2 chains (sqrt set once), then tails (gelu set once)
    for b in range(G):
        emit_rstd2(b, 0)
        emit_rstd2(b, 1)
    for b in range(G):
        emit_tail(b, 0)
        emit_tail(b, 1)


def build(kcs):
    from contextlib import ExitStack

    nc = bacc.Bacc("TRN2", target_bir_lowering=False, debug=False,
                   num_devices=NCORES)
    x = nc.dram_tensor("x", [G, N, D], BF16, kind="ExternalInput").ap()
    lb = nc.dram_tensor("lb", [G, N], F32, kind="ExternalInput").ap()
    wdram = {
        name: nc.dram_tensor(name, [D, D], BF16, kind="ExternalInput").ap()
        for name in ("wq", "wk", "wv", "wf", "wp")
    }
    out = nc.dram_tensor("out", [G, N, D], F32, kind="ExternalOutput").ap()
    warm = nc.dram_tensor("warm", [P, 8], F32, kind="ExternalOutput").ap()

    with tile.TileContext(nc) as tc:
        with ExitStack() as ctx:
            _body(ctx, tc, x, lb, wdram, out, warm, kcs)
    nc.compile()
    return nc


_NC_CACHE = {}


def get_nc(kcs):
    kcs = tuple(kcs)
    if kcs not in _NC_CACHE:
        _NC_CACHE[kcs] = build(kcs)
    return _NC_CACHE[kcs]


def compute_schedule(belief_base_sizes):
    """Sort batches by live-chunk count desc; slot s takes ranks [8s, 8s+8).

    Returns (order, kcs): order[s*8+c] = original batch index run on core c
    slot s; kcs[s] = max chunk count in slot s (compiled loop bound).
    """
    sizes = np.asarray(belief_base_sizes).astype(np.int64)
    nkc = (sizes + P - 1) // P
    nkc = np.clip(nkc, 1, NT)
    order = np.argsort(-nkc, kind="stable")
    kcs = tuple(int(nkc[order[s * NCORES]]) for s in range(G))
    return order, kcs


def make_in_maps(x, belief_base_sizes, g1, w_qkv, g2, w_fc, w_proj):
    x = np.asarray(x, dtype=np.float32)
    sizes = np.asarray(belief_base_sizes, dtype=np.int64)
    g1 = np.asarray(g1, dtype=np.float32)
    w_qkv = np.asarray(w_qkv, dtype=np.float32)
    g2 = np.asarray(g2, dtype=np.float32)
    w_fc = np.asarray(w_fc, dtype=np.float32)
    w_proj = np.asarray(w_proj, dtype=np.float32)

    bf = ml_dtypes.bfloat16
    wq = ((g1[:, None] * w_qkv[:, :D]) / np.float32(np.sqrt(D))).astype(bf)
    wk = (g1[:, None] * w_qkv[:, D:2 * D]).astype(bf)
    wv = (g1[:, None] * w_qkv[:, 2 * D:]).astype(bf)
    wf = (g2[:, None] * w_fc).astype(bf)
    wp = w_proj.astype(bf)

    lbias = np.where(np.arange(N)[None, :] < sizes[:, None],
                     np.float32(0.0), np.float32(NEG)).astype(np.float32)

    order, kcs = compute_schedule(sizes)
    xb = x.astype(bf)
    in_maps = []
    for c in range(NCORES):
        sel = [int(order[s * NCORES + c]) for s in range(G)]
        in_maps.append({
            "x": np.ascontiguousarray(xb[sel]),
            "lb": np.ascontiguousarray(lbias[sel]),
            "wq": np.ascontiguousarray(wq), "wk": np.ascontiguousarray(wk),
            "wv": np.ascontiguousarray(wv), "wf": np.ascontiguousarray(wf),
            "wp": np.ascontiguousarray(wp),
        })
    return in_maps, order, kcs


def kernel(x, belief_base_sizes, g1, w_qkv, g2, w_fc, w_proj):
    in_maps, order, kcs = make_in_maps(
        x, belief_base_sizes, g1, w_qkv, g2, w_fc, w_proj)
    nc = get_nc(kcs)
    res = run_bass_kernel_spmd(nc, in_maps, core_ids=list(range(NCORES)))
    out = np.empty((B, N, D), dtype=np.float32)
    for c in range(NCORES):
        for s in range(G):
            out[int(order[s * NCORES + c])] = res.results[c]["out"][s]
    return np.ascontiguousarray(out)
